# revision 14
# baseline (speedup 1.0000x reference)
"""FFT-based DCT-II on 8 trn2 NeuronCores (rev H, radix 128x32).

Per core (256 rows): Makhoul DCT->real-FFT, four-step radix-128x32.
Stage 1: 16 matmuls [K=128(n1), M=128 dense real-DFT slots, N=512],
one stationary, full-lane psum->sbuf casts split vector/scalar, rows
kept in the free dim (w=256). Mid-transpose via DRAM roundtrip with
clean descriptors both ways (writes multi-KB runs, reads 512B runs);
the t2 pair layout (upper K-half reversed-m via negative-stride reads)
makes stage 2 exactly 33 matmuls [K=128, M=128, N=256]. y fp16.

Schedule: x in 5 chunks (small first) ahead of everything on sync; hh
head early / tail late on scalar; 4 T-write groups (small last); reads
in 3 j-chunks; y-writes as 9 big DMAs on sync (idle in stage 2).
"""

import numpy as np

N = 4096
R = 2048
RPC = 256

_state = {}


def _tables():
    N1, N2 = 128, 32
    n1 = np.arange(N1)[:, None].astype(np.float64)
    jc = np.arange(65)[None, :].astype(np.float64)
    js = np.arange(1, 64)[None, :].astype(np.float64)
    F1c = np.cos(2 * np.pi * n1 * jc / N1)  # [128, 65]
    F1s = -np.sin(2 * np.pi * n1 * js / N1)  # [128, 63]
    w1_np = np.concatenate([F1c, F1s], axis=1).astype(np.float16)  # [128, 128]

    n2v = np.arange(N2)[:, None].astype(np.float64)
    k2v = np.arange(N2)[None, :].astype(np.float64)

    def HHs(k1):
        k = N1 * k2v + k1
        Gc = np.cos(2 * np.pi * n2v * k / N)
        Gs = -np.sin(2 * np.pi * n2v * k / N)
        cosE = np.cos(np.pi * k / (2 * N))
        sinE = np.sin(np.pi * k / (2 * N))
        sigma = 1.0 if k1 <= 64 else -1.0
        H1 = cosE * Gc + sinE * Gs
        H2 = sigma * (sinE * Gc - cosE * Gs)
        return np.concatenate([H1, H2], axis=0)  # [64, 32] rows (Bc n2, Bs n2)

    HH2 = np.zeros((33, 128, 128))
    for j in range(33):
        HH2[j][0:64, 0:32] = HHs(j)
        if 1 <= j <= 32:
            HH2[j][0:64, 32:64] = HHs(128 - j)
        if 0 <= j <= 31:
            HH2[j][64:128, 64:96] = HHs(64 - j)
        if 1 <= j <= 31:
            HH2[j][64:128, 96:128] = HHs(64 + j)
    # slots whose sin inputs are identically zero (memset on device)
    HH2[0][32:64, :] = 0.0
    HH2[0][96:128, :] = 0.0
    hh_np = HH2.transpose(1, 0, 2).astype(np.float16).copy()  # [128, 33, 128]

    # output slot -> k1 map: psum partitions (g, h, k2)
    k1map = np.full((33, 2, 2), -1, dtype=np.int64)
    for j in range(33):
        k1map[j, 0, 0] = j
        if 1 <= j <= 32:
            k1map[j, 0, 1] = 128 - j
        if 0 <= j <= 31:
            k1map[j, 1, 0] = 64 - j
        if 1 <= j <= 31:
            k1map[j, 1, 1] = 64 + j
    slot_of_k1 = np.empty(128, dtype=np.int64)
    for j in range(33):
        for g in range(2):
            for h in range(2):
                k1 = k1map[j, g, h]
                if 0 <= k1 < 128:
                    slot_of_k1[k1] = j * 4 + g * 2 + h
    return w1_np, hh_np, slot_of_k1


def _build():
    import concourse.tile as tile
    from concourse import bacc, mybir

    f16 = mybir.dt.float16
    f32 = mybir.dt.float32

    nc = bacc.Bacc("TRN2", target_bir_lowering=False, debug=False, num_devices=8)
    x1_d = nc.dram_tensor("x1", [128, 8192], f16, kind="ExternalInput").ap()
    w1_d = nc.dram_tensor("w1", [128, 128], f16, kind="ExternalInput").ap()
    hh_d = nc.dram_tensor("hh", [128, 33, 128], f16, kind="ExternalInput").ap()
    y_d = nc.dram_tensor("y", [33, 2, 2, 32, 256], f16, kind="ExternalOutput").ap()

    XCH = [(0, 2), (2, 5), (5, 8), (8, 12), (12, 16)]  # x chunks in f units
    GRP = [(0, 5), (5, 9), (9, 13), (13, 16)]  # T write groups in f units
    RCH = [(0, 6), (6, 18), (18, 33)]  # read chunks in j units

    with tile.TileContext(nc) as tc:
        with (
            tc.tile_pool(name="const", bufs=1) as const,
            tc.tile_pool(name="data", bufs=1) as data,
            tc.tile_pool(name="dram", bufs=1, space="DRAM") as dram,
            tc.tile_pool(name="ps1", bufs=3, space="PSUM") as ps1,
            tc.tile_pool(name="ps2", bufs=4, space="PSUM") as ps2,
            tc.tile_pool(name="ysb", bufs=4) as ysb,
        ):
            w1_sb = const.tile([128, 128], f16)
            hh_sb = const.tile([128, 33, 128], f16)
            x1_sb = data.tile([128, 8192], f16)
            t_sb = data.tile([128, 32, 256], f16)  # slot x n2 x w
            t2_sb = data.tile([128, 33, 256], f16)  # (blk,c,n2) x j x w
            t_dram = dram.tile([128, 32, 256], f16)

            # zero sin slots with no source: lowBs j=0, upBs j=0
            nc.gpsimd.memset(t2_sb[32:64, 0, :], 0.0)
            nc.gpsimd.memset(t2_sb[96:128, 0, :], 0.0)

            # x chunks ahead of everything on sync; w1 + hh head on scalar
            for fa, fb in XCH:
                sl = slice(512 * fa, 512 * fb)
                nc.sync.dma_start(x1_sb[:, sl], x1_d[:, sl])
            nc.scalar.dma_start(w1_sb[:], w1_d)
            nc.scalar.dma_start(hh_sb[:, 0:6, :], hh_d[:, 0:6, :])

            # stage 1
            gi = 0
            for f in range(16):
                ps = ps1.tile([128, 512], f32, name=f"s1_{f}", tag="s1ps")
                nc.tensor.matmul(
                    ps[:],
                    w1_sb[:],
                    x1_sb[:, 512 * f : 512 * f + 512],
                    start=True,
                    stop=True,
                )
                dst = t_sb[:, 2 * f : 2 * f + 2, :]
                src = ps[:].rearrange("p (n w) -> p n w", n=2)
                if f % 3 == 2:
                    nc.scalar.copy(dst, src)
                else:
                    nc.vector.tensor_copy(dst, src)

                if f == 8:
                    # WAW chain: pin the hh tail loads behind stage-1
                    # progress so the scheduler can't hoist them into the
                    # x-load window (they'd steal HBM bandwidth from x).
                    nc.gpsimd.tensor_copy(
                        hh_sb[0:32, 6:7, 0:1], t_sb[0:32, 16:17, 0:1]
                    )
                    nc.gpsimd.tensor_copy(
                        hh_sb[0:32, 20:21, 0:1], t_sb[0:32, 16:17, 0:1]
                    )
                    nc.scalar.dma_start(hh_sb[:, 6:20, :], hh_d[:, 6:20, :])
                    nc.scalar.dma_start(hh_sb[:, 20:33, :], hh_d[:, 20:33, :])

                if gi < len(GRP) and f == GRP[gi][1] - 1:
                    fa, fb = GRP[gi]
                    n2a, n2b = 2 * fa, 2 * fb
                    nc.sync.dma_start(t_dram[:, n2a:n2b, :], t_sb[:, n2a:n2b, :])
                    gi += 1

            # T reads: dst partitions = n2, free = (j, w); src rows = slots.
            # lowBc: s=j ; lowBs: s=64+j ; upBc: s=64-j ; upBs: s=128-j
            def rd(par0, ja, jb, srows, eng):
                eng.dma_start(
                    t2_sb[par0 : par0 + 32, ja:jb, :],
                    t_dram[srows, :, :].rearrange("s n w -> n s w"),
                )

            for ja, jb in RCH:
                rd(0, ja, jb, slice(ja, jb), nc.sync)
                rd(64, ja, jb, slice(64 - ja, 64 - jb, -1), nc.sync)
                ja1 = max(ja, 1)
                rd(32, ja1, jb, slice(64 + ja1, 64 + jb), nc.scalar)
                rd(96, ja1, jb, slice(128 - ja1, 128 - jb, -1), nc.scalar)

            # stage 2: 33 matmuls, pair weights; 17 psum tiles of <=2 j;
            # y staged in [128,1024] tiles (4 j) written on sync
            ytile = None
            for q in range(17):
                nj = 2 if q < 16 else 1
                ps = ps2.tile([128, 512], f32, name=f"s2_{q}", tag="s2ps")
                for i in range(nj):
                    j = 2 * q + i
                    nc.tensor.matmul(
                        ps[:, 256 * i : 256 * i + 256],
                        hh_sb[:, j, :],
                        t2_sb[:, j, :],
                        start=True,
                        stop=True,
                    )
                if q % 2 == 0:
                    ytile = ysb.tile([128, 1024], f16, name=f"y_{q//2}", tag="ysb")
                cp_dst = ytile[:, 512 * (q % 2) : 512 * (q % 2) + 256 * nj]
                cp_src = ps[:, 0 : 256 * nj]
                if q % 2 == 0:
                    nc.vector.tensor_copy(cp_dst, cp_src)
                else:
                    nc.scalar.copy(cp_dst, cp_src)
                if q % 2 == 1 or q == 16:
                    j0 = 4 * (q // 2)
                    njj = 4 if q % 2 == 1 else 1
                    ydst = y_d[j0 : j0 + njj].rearrange("j g h k w -> (g h k) j w")
                    ysrc = ytile[:, 0 : 256 * njj].rearrange(
                        "p (j w) -> p j w", w=256
                    )
                    if (q // 2) % 2 == 0:
                        nc.gpsimd.dma_start(ydst, ysrc)
                    else:
                        nc.sync.dma_start(ydst, ysrc)

    nc.compile()
    return nc


def _pack_x1(x_rows):
    v = np.empty_like(x_rows)
    v[:, : N // 2] = x_rows[:, 0::2]
    v[:, N // 2 :] = x_rows[:, 1::2][:, ::-1]
    # x1[n1, n2, r] = v[r, 32*n1 + n2]
    x1 = v.reshape(RPC, 128, 32).transpose(1, 2, 0).reshape(128, 8192)
    return np.ascontiguousarray(x1.astype(np.float16))


def kernel(x, _trace: bool = False):
    from concourse.bass_utils import run_bass_kernel_spmd

    x = np.asarray(x, dtype=np.float32)
    assert x.shape == (R, N)
    if "nc" not in _state:
        _state["nc"] = _build()
        _state["tables"] = _tables()
    nc = _state["nc"]
    w1_np, hh_np, slot_of_k1 = _state["tables"]

    in_maps = []
    for c in range(8):
        in_maps.append(
            {
                "x1": _pack_x1(x[c * RPC : (c + 1) * RPC]),
                "w1": w1_np,
                "hh": hh_np,
            }
        )

    res = run_bass_kernel_spmd(nc, in_maps, list(range(8)), trace=_trace)

    y = np.empty((R, N), dtype=np.float32)
    for c in range(8):
        ydev = res.results[c]["y"].astype(np.float32)  # [33, 2, 2, 32, 256]
        yk = ydev.transpose(4, 3, 0, 1, 2).reshape(RPC, 32, 132)
        y[c * RPC : (c + 1) * RPC] = yk[:, :, slot_of_k1].reshape(RPC, N)
    if _trace:
        _state["last_result"] = res
    return y


# revision 18
# speedup vs baseline: 1.0064x; 1.0064x over previous
"""FFT-based DCT-II on 8 trn2 NeuronCores (rev H, radix 128x32).

Per core (256 rows): Makhoul DCT->real-FFT, four-step radix-128x32.
Stage 1: 16 matmuls [K=128(n1), M=128 dense real-DFT slots, N=512],
one stationary, full-lane psum->sbuf casts split vector/scalar, rows
kept in the free dim (w=256). Mid-transpose via DRAM roundtrip with
clean descriptors both ways (writes multi-KB runs, reads 512B runs);
the t2 pair layout (upper K-half reversed-m via negative-stride reads)
makes stage 2 exactly 33 matmuls [K=128, M=128, N=256]. y fp16.

Schedule: x in 5 chunks (small first) ahead of everything on sync; hh
head early / tail late on scalar; 4 T-write groups (small last); reads
in 3 j-chunks; y-writes as 9 big DMAs on sync (idle in stage 2).
"""

import numpy as np

N = 4096
R = 2048
RPC = 256

_state = {}


def _tables():
    N1, N2 = 128, 32
    n1 = np.arange(N1)[:, None].astype(np.float64)
    jc = np.arange(65)[None, :].astype(np.float64)
    js = np.arange(1, 64)[None, :].astype(np.float64)
    F1c = np.cos(2 * np.pi * n1 * jc / N1)  # [128, 65]
    F1s = -np.sin(2 * np.pi * n1 * js / N1)  # [128, 63]
    w1_np = np.concatenate([F1c, F1s], axis=1).astype(np.float16)  # [128, 128]

    n2v = np.arange(N2)[:, None].astype(np.float64)
    k2v = np.arange(N2)[None, :].astype(np.float64)

    def HHs(k1):
        k = N1 * k2v + k1
        Gc = np.cos(2 * np.pi * n2v * k / N)
        Gs = -np.sin(2 * np.pi * n2v * k / N)
        cosE = np.cos(np.pi * k / (2 * N))
        sinE = np.sin(np.pi * k / (2 * N))
        sigma = 1.0 if k1 <= 64 else -1.0
        H1 = cosE * Gc + sinE * Gs
        H2 = sigma * (sinE * Gc - cosE * Gs)
        return np.concatenate([H1, H2], axis=0)  # [64, 32] rows (Bc n2, Bs n2)

    HH2 = np.zeros((33, 128, 128))
    for j in range(33):
        HH2[j][0:64, 0:32] = HHs(j)
        if 1 <= j <= 32:
            HH2[j][0:64, 32:64] = HHs(128 - j)
        if 0 <= j <= 31:
            HH2[j][64:128, 64:96] = HHs(64 - j)
        if 1 <= j <= 31:
            HH2[j][64:128, 96:128] = HHs(64 + j)
    # slots whose sin inputs are identically zero (memset on device)
    HH2[0][32:64, :] = 0.0
    HH2[0][96:128, :] = 0.0
    hh_np = HH2.transpose(1, 0, 2).astype(np.float16).copy()  # [128, 33, 128]

    # output slot -> k1 map: psum partitions (g, h, k2)
    k1map = np.full((33, 2, 2), -1, dtype=np.int64)
    for j in range(33):
        k1map[j, 0, 0] = j
        if 1 <= j <= 32:
            k1map[j, 0, 1] = 128 - j
        if 0 <= j <= 31:
            k1map[j, 1, 0] = 64 - j
        if 1 <= j <= 31:
            k1map[j, 1, 1] = 64 + j
    slot_of_k1 = np.empty(128, dtype=np.int64)
    for j in range(33):
        for g in range(2):
            for h in range(2):
                k1 = k1map[j, g, h]
                if 0 <= k1 < 128:
                    slot_of_k1[k1] = j * 4 + g * 2 + h
    return w1_np, hh_np, slot_of_k1


def _build():
    import concourse.tile as tile
    from concourse import bacc, mybir

    f16 = mybir.dt.float16
    f32 = mybir.dt.float32

    nc = bacc.Bacc("TRN2", target_bir_lowering=False, debug=False, num_devices=8)
    x1_d = nc.dram_tensor("x1", [128, 8192], f16, kind="ExternalInput").ap()
    w1_d = nc.dram_tensor("w1", [128, 128], f16, kind="ExternalInput").ap()
    hh_d = nc.dram_tensor("hh", [128, 33, 128], f16, kind="ExternalInput").ap()
    y_d = nc.dram_tensor("y", [33, 2, 2, 32, 256], f16, kind="ExternalOutput").ap()

    XCH = [(0, 2), (2, 5), (5, 8), (8, 11), (11, 14), (14, 16)]  # x chunks (f)
    GRP = [(0, 5), (5, 9), (9, 12), (12, 14), (14, 16)]  # T write groups (f)
    RCH = [(0, 8), (8, 20), (20, 33)]  # read chunks in j units

    with tile.TileContext(nc) as tc:
        with (
            tc.tile_pool(name="const", bufs=1) as const,
            tc.tile_pool(name="data", bufs=1) as data,
            tc.tile_pool(name="dram", bufs=1, space="DRAM") as dram,
            tc.tile_pool(name="ps1", bufs=3, space="PSUM") as ps1,
            tc.tile_pool(name="ps2", bufs=4, space="PSUM") as ps2,
            tc.tile_pool(name="ysb", bufs=6) as ysb,
        ):
            w1_sb = const.tile([128, 128], f16)
            hh_sb = const.tile([128, 33, 128], f16)
            x1_sb = data.tile([128, 8192], f16)
            t_sb = data.tile([128, 32, 256], f16)  # slot x n2 x w
            t2_sb = data.tile([128, 33, 256], f16)  # (blk,c,n2) x j x w
            t_dram = dram.tile([128, 32, 256], f16)

            # zero sin slots with no source: lowBs j=0, upBs j=0
            nc.gpsimd.memset(t2_sb[32:64, 0, :], 0.0)
            nc.gpsimd.memset(t2_sb[96:128, 0, :], 0.0)

            # x chunks ahead of everything on sync; w1 + hh head on scalar
            for fa, fb in XCH:
                sl = slice(512 * fa, 512 * fb)
                nc.sync.dma_start(x1_sb[:, sl], x1_d[:, sl])
            nc.scalar.dma_start(w1_sb[:], w1_d)
            nc.scalar.dma_start(hh_sb[:, 0:6, :], hh_d[:, 0:6, :])

            # stage 1
            gi = 0
            for f in range(16):
                ps = ps1.tile([128, 512], f32, name=f"s1_{f}", tag="s1ps")
                nc.tensor.matmul(
                    ps[:],
                    w1_sb[:],
                    x1_sb[:, 512 * f : 512 * f + 512],
                    start=True,
                    stop=True,
                )
                dst = t_sb[:, 2 * f : 2 * f + 2, :]
                src = ps[:].rearrange("p (n w) -> p n w", n=2)
                if f % 3 == 2:
                    nc.scalar.copy(dst, src)
                else:
                    nc.vector.tensor_copy(dst, src)

                if f == 8:
                    # WAW chain: pin the hh tail loads behind stage-1
                    # progress so the scheduler can't hoist them into the
                    # x-load window (they'd steal HBM bandwidth from x).
                    nc.gpsimd.tensor_copy(
                        hh_sb[0:32, 6:7, 0:1], t_sb[0:32, 16:17, 0:1]
                    )
                    nc.gpsimd.tensor_copy(
                        hh_sb[0:32, 20:21, 0:1], t_sb[0:32, 16:17, 0:1]
                    )
                    nc.scalar.dma_start(hh_sb[:, 6:20, :], hh_d[:, 6:20, :])
                    nc.scalar.dma_start(hh_sb[:, 20:33, :], hh_d[:, 20:33, :])

                if gi < len(GRP) and f == GRP[gi][1] - 1:
                    fa, fb = GRP[gi]
                    n2a, n2b = 2 * fa, 2 * fb
                    tw_eng = nc.sync if gi % 2 == 0 else nc.scalar
                    tw_eng.dma_start(t_dram[:, n2a:n2b, :], t_sb[:, n2a:n2b, :])
                    gi += 1

            # T reads: dst partitions = n2, free = (j, w); src rows = slots.
            # lowBc: s=j ; lowBs: s=64+j ; upBc: s=64-j ; upBs: s=128-j
            def rd(par0, ja, jb, srows, eng):
                eng.dma_start(
                    t2_sb[par0 : par0 + 32, ja:jb, :],
                    t_dram[srows, :, :].rearrange("s n w -> n s w"),
                )

            for ja, jb in RCH:
                rd(0, ja, jb, slice(ja, jb), nc.sync)
                rd(64, ja, jb, slice(64 - ja, 64 - jb, -1), nc.sync)
                ja1 = max(ja, 1)
                rd(32, ja1, jb, slice(64 + ja1, 64 + jb), nc.scalar)
                rd(96, ja1, jb, slice(128 - ja1, 128 - jb, -1), nc.scalar)

            # stage 2: 33 matmuls, pair weights; 17 psum tiles of <=2 j;
            # y staged in [128,1024] tiles (4 j) written on sync
            ytile = None
            for q in range(17):
                nj = 2 if q < 16 else 1
                ps = ps2.tile([128, 512], f32, name=f"s2_{q}", tag="s2ps")
                for i in range(nj):
                    j = 2 * q + i
                    nc.tensor.matmul(
                        ps[:, 256 * i : 256 * i + 256],
                        hh_sb[:, j, :],
                        t2_sb[:, j, :],
                        start=True,
                        stop=True,
                    )
                if q % 2 == 0:
                    ytile = ysb.tile([128, 1024], f16, name=f"y_{q//2}", tag="ysb")
                cp_dst = ytile[:, 512 * (q % 2) : 512 * (q % 2) + 256 * nj]
                cp_src = ps[:, 0 : 256 * nj]
                if q % 2 == 0:
                    nc.vector.tensor_copy(cp_dst, cp_src)
                else:
                    nc.scalar.copy(cp_dst, cp_src)
                if q % 2 == 1 or q == 16:
                    j0 = 4 * (q // 2)
                    njj = 4 if q % 2 == 1 else 1
                    ydst = y_d[j0 : j0 + njj].rearrange("j g h k w -> (g h k) j w")
                    ysrc = ytile[:, 0 : 256 * njj].rearrange(
                        "p (j w) -> p j w", w=256
                    )
                    yeng = [nc.sync, nc.scalar, nc.gpsimd][(q // 2) % 3]
                    yeng.dma_start(ydst, ysrc)

    nc.compile()
    return nc


def _pack_x1(x_rows):
    v = np.empty_like(x_rows)
    v[:, : N // 2] = x_rows[:, 0::2]
    v[:, N // 2 :] = x_rows[:, 1::2][:, ::-1]
    # x1[n1, n2, r] = v[r, 32*n1 + n2]
    x1 = v.reshape(RPC, 128, 32).transpose(1, 2, 0).reshape(128, 8192)
    return np.ascontiguousarray(x1.astype(np.float16))


def kernel(x, _trace: bool = False):
    from concourse.bass_utils import run_bass_kernel_spmd

    x = np.asarray(x, dtype=np.float32)
    assert x.shape == (R, N)
    if "nc" not in _state:
        _state["nc"] = _build()
        _state["tables"] = _tables()
    nc = _state["nc"]
    w1_np, hh_np, slot_of_k1 = _state["tables"]

    in_maps = []
    for c in range(8):
        in_maps.append(
            {
                "x1": _pack_x1(x[c * RPC : (c + 1) * RPC]),
                "w1": w1_np,
                "hh": hh_np,
            }
        )

    res = run_bass_kernel_spmd(nc, in_maps, list(range(8)), trace=_trace)

    y = np.empty((R, N), dtype=np.float32)
    for c in range(8):
        ydev = res.results[c]["y"].astype(np.float32)  # [33, 2, 2, 32, 256]
        yk = ydev.transpose(4, 3, 0, 1, 2).reshape(RPC, 32, 132)
        y[c * RPC : (c + 1) * RPC] = yk[:, :, slot_of_k1].reshape(RPC, N)
    if _trace:
        _state["last_result"] = res
    return y


# revision 22
# speedup vs baseline: 1.0182x; 1.0117x over previous
"""FFT-based DCT-II on 8 trn2 NeuronCores (rev H, radix 128x32).

Per core (256 rows): Makhoul DCT->real-FFT, four-step radix-128x32.
Stage 1: 16 matmuls [K=128(n1), M=128 dense real-DFT slots, N=512],
one stationary, full-lane psum->sbuf casts split vector/scalar, rows
kept in the free dim (w=256). Mid-transpose via DRAM roundtrip with
clean descriptors both ways (writes multi-KB runs, reads 512B runs);
the t2 pair layout (upper K-half reversed-m via negative-stride reads)
makes stage 2 exactly 33 matmuls [K=128, M=128, N=256]. y fp16.

Schedule: x in 5 chunks (small first) ahead of everything on sync; hh
head early / tail late on scalar; 4 T-write groups (small last); reads
in 3 j-chunks; y-writes as 9 big DMAs on sync (idle in stage 2).
"""

import numpy as np

N = 4096
R = 2048
RPC = 256

_state = {}


def _tables():
    N1, N2 = 128, 32
    n1 = np.arange(N1)[:, None].astype(np.float64)
    jc = np.arange(65)[None, :].astype(np.float64)
    js = np.arange(1, 64)[None, :].astype(np.float64)
    F1c = np.cos(2 * np.pi * n1 * jc / N1)  # [128, 65]
    F1s = -np.sin(2 * np.pi * n1 * js / N1)  # [128, 63]
    w1_np = np.concatenate([F1c, F1s], axis=1).astype(np.float16)  # [128, 128]

    n2v = np.arange(N2)[:, None].astype(np.float64)
    k2v = np.arange(N2)[None, :].astype(np.float64)

    def HHs(k1):
        k = N1 * k2v + k1
        Gc = np.cos(2 * np.pi * n2v * k / N)
        Gs = -np.sin(2 * np.pi * n2v * k / N)
        cosE = np.cos(np.pi * k / (2 * N))
        sinE = np.sin(np.pi * k / (2 * N))
        sigma = 1.0 if k1 <= 64 else -1.0
        H1 = cosE * Gc + sinE * Gs
        H2 = sigma * (sinE * Gc - cosE * Gs)
        return np.concatenate([H1, H2], axis=0)  # [64, 32] rows (Bc n2, Bs n2)

    HH2 = np.zeros((33, 128, 128))
    for j in range(33):
        HH2[j][0:64, 0:32] = HHs(j)
        if 1 <= j <= 32:
            HH2[j][0:64, 32:64] = HHs(128 - j)
        if 0 <= j <= 31:
            HH2[j][64:128, 64:96] = HHs(64 - j)
        if 1 <= j <= 31:
            HH2[j][64:128, 96:128] = HHs(64 + j)
    # slots whose sin inputs are identically zero (memset on device)
    HH2[0][32:64, :] = 0.0
    HH2[0][96:128, :] = 0.0
    hh_np = HH2.transpose(1, 0, 2).astype(np.float16).copy()  # [128, 33, 128]

    # output slot -> k1 map: psum partitions (g, h, k2)
    k1map = np.full((33, 2, 2), -1, dtype=np.int64)
    for j in range(33):
        k1map[j, 0, 0] = j
        if 1 <= j <= 32:
            k1map[j, 0, 1] = 128 - j
        if 0 <= j <= 31:
            k1map[j, 1, 0] = 64 - j
        if 1 <= j <= 31:
            k1map[j, 1, 1] = 64 + j
    slot_of_k1 = np.empty(128, dtype=np.int64)
    for j in range(33):
        for g in range(2):
            for h in range(2):
                k1 = k1map[j, g, h]
                if 0 <= k1 < 128:
                    slot_of_k1[k1] = j * 4 + g * 2 + h
    return w1_np, hh_np, slot_of_k1


def _build():
    import concourse.tile as tile
    from concourse import bacc, mybir

    f16 = mybir.dt.float16
    f32 = mybir.dt.float32

    nc = bacc.Bacc("TRN2", target_bir_lowering=False, debug=False, num_devices=8)
    x1_d = nc.dram_tensor("x1", [128, 8192], f16, kind="ExternalInput").ap()
    w1_d = nc.dram_tensor("w1", [128, 128], f16, kind="ExternalInput").ap()
    hh_d = nc.dram_tensor("hh", [128, 33, 128], f16, kind="ExternalInput").ap()
    # y layout: partition-major so y writes are 2KB-contiguous per partition
    y_d = nc.dram_tensor("y", [128, 33, 256], f16, kind="ExternalOutput").ap()

    XCH = [(0, 2), (2, 5), (5, 8), (8, 11), (11, 14), (14, 16)]  # x chunks (f)
    GRP = [(0, 5), (5, 9), (9, 12), (12, 14), (14, 16)]  # T write groups (f)
    RCH = [(0, 8), (8, 20), (20, 33)]  # read chunks in j units

    with tile.TileContext(nc) as tc:
        with (
            tc.tile_pool(name="const", bufs=1) as const,
            tc.tile_pool(name="data", bufs=1) as data,
            tc.tile_pool(name="dram", bufs=1, space="DRAM") as dram,
            tc.tile_pool(name="ps1", bufs=3, space="PSUM") as ps1,
            tc.tile_pool(name="ps2", bufs=4, space="PSUM") as ps2,
            tc.tile_pool(name="ysb", bufs=6) as ysb,
        ):
            w1_sb = const.tile([128, 128], f16)
            hh_sb = const.tile([128, 33, 128], f16)
            x1_sb = data.tile([128, 8192], f16)
            t_sb = data.tile([128, 32, 256], f16)  # slot x n2 x w
            t2_sb = data.tile([128, 33, 256], f16)  # (blk,c,n2) x j x w
            t_dram = dram.tile([128, 32, 256], f16)

            # zero sin slots with no source: lowBs j=0, upBs j=0
            nc.gpsimd.memset(t2_sb[32:64, 0, :], 0.0)
            nc.gpsimd.memset(t2_sb[96:128, 0, :], 0.0)

            # primer: absorb the SDMA first-transfer ramp before x lands
            prime_sb = data.tile([128, 16], f16)
            nc.sync.dma_start(prime_sb[:], x1_d[:, 0:16])

            # x chunks ahead of everything on sync; w1 + hh head on scalar
            for fa, fb in XCH:
                sl = slice(512 * fa, 512 * fb)
                nc.sync.dma_start(x1_sb[:, sl], x1_d[:, sl])
            nc.scalar.dma_start(w1_sb[:], w1_d)
            nc.scalar.dma_start(hh_sb[:, 0:6, :], hh_d[:, 0:6, :])

            # stage 1
            gi = 0
            for f in range(16):
                ps = ps1.tile([128, 512], f32, name=f"s1_{f}", tag="s1ps")
                nc.tensor.matmul(
                    ps[:],
                    w1_sb[:],
                    x1_sb[:, 512 * f : 512 * f + 512],
                    start=True,
                    stop=True,
                )
                dst = t_sb[:, 2 * f : 2 * f + 2, :]
                src = ps[:].rearrange("p (n w) -> p n w", n=2)
                if f % 3 == 2:
                    nc.scalar.copy(dst, src)
                else:
                    nc.vector.tensor_copy(dst, src)

                if f == 8:
                    # WAW chain: pin the hh tail loads behind stage-1
                    # progress so the scheduler can't hoist them into the
                    # x-load window (they'd steal HBM bandwidth from x).
                    nc.gpsimd.tensor_copy(
                        hh_sb[0:32, 6:7, 0:1], t_sb[0:32, 16:17, 0:1]
                    )
                    nc.gpsimd.tensor_copy(
                        hh_sb[0:32, 20:21, 0:1], t_sb[0:32, 16:17, 0:1]
                    )
                    nc.scalar.dma_start(hh_sb[:, 6:20, :], hh_d[:, 6:20, :])
                    nc.scalar.dma_start(hh_sb[:, 20:33, :], hh_d[:, 20:33, :])

                if gi < len(GRP) and f == GRP[gi][1] - 1:
                    fa, fb = GRP[gi]
                    n2a, n2b = 2 * fa, 2 * fb
                    tw_eng = nc.sync if gi % 2 == 0 else nc.scalar
                    tw_eng.dma_start(t_dram[:, n2a:n2b, :], t_sb[:, n2a:n2b, :])
                    gi += 1

            # T reads: dst partitions = n2, free = (j, w); src rows = slots.
            # lowBc: s=j ; lowBs: s=64+j ; upBc: s=64-j ; upBs: s=128-j
            def rd(par0, ja, jb, srows, eng):
                eng.dma_start(
                    t2_sb[par0 : par0 + 32, ja:jb, :],
                    t_dram[srows, :, :].rearrange("s n w -> n s w"),
                )

            for ja, jb in RCH:
                rd(0, ja, jb, slice(ja, jb), nc.sync)
                rd(64, ja, jb, slice(64 - ja, 64 - jb, -1), nc.sync)
                ja1 = max(ja, 1)
                rd(32, ja1, jb, slice(64 + ja1, 64 + jb), nc.scalar)
                rd(96, ja1, jb, slice(128 - ja1, 128 - jb, -1), nc.scalar)

            # stage 2: 33 matmuls, pair weights; 17 psum tiles of <=2 j;
            # y staged in [128,1024] tiles (4 j) written on sync
            ytile = None
            for q in range(17):
                nj = 2 if q < 16 else 1
                ps = ps2.tile([128, 512], f32, name=f"s2_{q}", tag="s2ps")
                for i in range(nj):
                    j = 2 * q + i
                    nc.tensor.matmul(
                        ps[:, 256 * i : 256 * i + 256],
                        hh_sb[:, j, :],
                        t2_sb[:, j, :],
                        start=True,
                        stop=True,
                    )
                if q % 2 == 0:
                    ytile = ysb.tile([128, 1024], f16, name=f"y_{q//2}", tag="ysb")
                cp_dst = ytile[:, 512 * (q % 2) : 512 * (q % 2) + 256 * nj]
                cp_src = ps[:, 0 : 256 * nj]
                if q % 2 == 0:
                    nc.vector.tensor_copy(cp_dst, cp_src)
                else:
                    nc.scalar.copy(cp_dst, cp_src)
                if q % 2 == 1 or q == 16:
                    j0 = 4 * (q // 2)
                    njj = 4 if q % 2 == 1 else 1
                    ydst = y_d[:, j0 : j0 + njj, :]
                    ysrc = ytile[:, 0 : 256 * njj].rearrange(
                        "p (j w) -> p j w", w=256
                    )
                    yeng = [nc.sync, nc.scalar, nc.gpsimd][(q // 2) % 3]
                    yeng.dma_start(ydst, ysrc)

    nc.compile()
    return nc


def _pack_x1(x_rows):
    v = np.empty_like(x_rows)
    v[:, : N // 2] = x_rows[:, 0::2]
    v[:, N // 2 :] = x_rows[:, 1::2][:, ::-1]
    # x1[n1, n2, r] = v[r, 32*n1 + n2]
    x1 = v.reshape(RPC, 128, 32).transpose(1, 2, 0).reshape(128, 8192)
    return np.ascontiguousarray(x1.astype(np.float16))


def kernel(x, _trace: bool = False):
    from concourse.bass_utils import run_bass_kernel_spmd

    x = np.asarray(x, dtype=np.float32)
    assert x.shape == (R, N)
    if "nc" not in _state:
        _state["nc"] = _build()
        _state["tables"] = _tables()
    nc = _state["nc"]
    w1_np, hh_np, slot_of_k1 = _state["tables"]

    in_maps = []
    for c in range(8):
        in_maps.append(
            {
                "x1": _pack_x1(x[c * RPC : (c + 1) * RPC]),
                "w1": w1_np,
                "hh": hh_np,
            }
        )

    res = run_bass_kernel_spmd(nc, in_maps, list(range(8)), trace=_trace)

    # decode: ydev[p=(g,h,k2), j, w];  y[w, 128*k2 + k1] = ydev[g*64+h*32+k2, j, w]
    jj = slot_of_k1 // 4
    gg = (slot_of_k1 % 4) // 2
    hh = slot_of_k1 % 2
    k2i = np.arange(32)
    part_idx = gg[None, :] * 64 + hh[None, :] * 32 + k2i[:, None]  # [32, 128]
    jj_b = np.broadcast_to(jj[None, :], (32, 128))
    y = np.empty((R, N), dtype=np.float32)
    for c in range(8):
        ydev = res.results[c]["y"].astype(np.float32)  # [128, 33, 256]
        yc = ydev[part_idx, jj_b, :]  # [32, 128, 256]
        y[c * RPC : (c + 1) * RPC] = yc.transpose(2, 0, 1).reshape(RPC, N)
    if _trace:
        _state["last_result"] = res
    return y
